# revision 19
# baseline (speedup 1.0000x reference)
"""NeRF volumetric alpha-compositing kernel for Trainium2 (Bass/Tile).

Full inputs:  rgbo [131072, 128, 4] f32, depth [131072, 128] f32.
Full output:  [131072, 3] f32.

Sharding: data-parallel over rays, 8 cores x 16384 rays.

Per-core algorithm (ray-per-partition layout; S=128 samples on free dim):
  delta[s]  = depth[s+1] - depth[s]            (DVE, shifted views)
  m[s]      = opacity[s] * delta[s]            (DVE; m[127] = opacity*1e9)
  cs        = inclusive_cumsum(m)              (DVE tensor_tensor_scan)
  t[0]      = 1;  t[i] = exp(-cs[i-1])         (ScalarE Exp, scale=-1)
  w[i]      = t[i] - t[i+1]                    (DVE)   [= T_i * alpha_i]
  out[c]    = sum_s w[s] * sigmoid(rgb[s,c])   (DVE tensor_tensor_reduce)
The last-sample FAR_DELTA=1e9 is exact: t[128]=exp(-cs[127]) underflows to 0
whenever opacity[127] > ~1e-7, else matches the reference expression.
"""

from contextlib import ExitStack

import numpy as np

import concourse.bass as bass
import concourse.tile as tile
from concourse import bacc, mybir
from concourse.bass_utils import run_bass_kernel_spmd

N_RAYS = 131072
S = 128
N_CORES = 8
NC_RAYS = N_RAYS // N_CORES  # 16384 rays per core
BLOCK = 128                  # rays per partition-block
F32 = mybir.dt.float32
BF16 = mybir.dt.bfloat16


def build_nerf_bass(
    n_rays: int = NC_RAYS,
    t_blocks: int = 8,
    gpsimd_delta_m: bool = False,
    gpsimd_channels: int = 0,
    repeat: int = 1,
    dma_only: bool = False,
    loop_iters: int = 0,
    skip: tuple = (),
    bufs: int = 2,
    scr_bufs: int = 4,
    dma_split: bool = False,
) -> bass.Bass:
    """Build the per-core Bass program for n_rays rays.

    gpsimd_delta_m: compute delta and m on GPSIMD instead of VectorE.
    gpsimd_channels: how many of the 3 weighted-reduce channels run as
      fused scalar_tensor_tensor(+accum) on GPSIMD instead of
      tensor_tensor_reduce on VectorE.
    """
    T = t_blocks
    SUPER = BLOCK * T
    assert n_rays % SUPER == 0
    n_super = n_rays // SUPER
    U = S + 4  # padded per-block stride for the t-table (129 used)

    nc = bacc.Bacc("TRN2", target_bir_lowering=False, debug=False)
    rgbo_h = nc.declare_dram_parameter("rgbo", [n_rays, S, 4], F32, isOutput=False)
    depth_h = nc.declare_dram_parameter("depth", [n_rays, S], F32, isOutput=False)
    out_h = nc.declare_dram_parameter("out", [n_rays, 3], F32, isOutput=True)

    rgbo_ap = rgbo_h.ap()
    depth_ap = depth_h.ap()
    out_ap = out_h.ap()

    with ExitStack() as ctx:
        tc = ctx.enter_context(tile.TileContext(nc))
        p_rgbo = ctx.enter_context(tc.tile_pool(name="rgbo", bufs=bufs))
        p_depth = ctx.enter_context(tc.tile_pool(name="depth", bufs=bufs))
        p_g = ctx.enter_context(tc.tile_pool(name="g", bufs=bufs))
        p_mid = ctx.enter_context(tc.tile_pool(name="mid", bufs=bufs))
        p_scr = ctx.enter_context(tc.tile_pool(name="scr", bufs=scr_bufs))
        p_out = ctx.enter_context(tc.tile_pool(name="outp", bufs=bufs))

        def emit_superblock(r0):
            rgbo_t = p_rgbo.tile([BLOCK, 4 * S * T], F32, tag="rgbo")
            rgbo_dst = rgbo_t.rearrange("p (t f) -> p t f", t=T)
            rgbo_src = rgbo_ap[r0 : r0 + SUPER].rearrange(
                "(p t) s c -> p t (s c)", p=BLOCK
            )
            depth_t = p_depth.tile([BLOCK, S * T], F32, tag="depth")
            depth_dst = depth_t.rearrange("p (t s) -> p t s", t=T)
            depth_src = depth_ap[r0 : r0 + SUPER].rearrange(
                "(p t) s -> p t s", p=BLOCK
            )
            if dma_split:
                h = T // 2
                nc.sync.dma_start(out=rgbo_dst[:, 0:h], in_=rgbo_src[:, 0:h])
                nc.scalar.dma_start(out=rgbo_dst[:, h:T], in_=rgbo_src[:, h:T])
                nc.gpsimd.dma_start(out=depth_dst, in_=depth_src)
            else:
                nc.sync.dma_start(out=rgbo_dst, in_=rgbo_src)
                nc.sync.dma_start(out=depth_dst, in_=depth_src)
            rgbo4 = rgbo_t.rearrange("p (t s c) -> p t s c", t=T, s=S, c=4)
            depth3 = depth_t.rearrange("p (t s) -> p t s", t=T)

            if dma_only:
                # consume both loads (prevents DCE), write output, skip compute
                out_t = p_out.tile([BLOCK, 3 * T], F32, tag="out")
                nc.vector.scalar_tensor_tensor(
                    out=out_t[:, 0:1], in0=rgbo_t[:, 0:1], scalar=0.0,
                    in1=depth_t[:, 0:1], op0=mybir.AluOpType.mult,
                    op1=mybir.AluOpType.add,
                )
                nc.vector.memset(out_t[:, 1 : 3 * T], 0.0)
                (nc.gpsimd if dma_split else nc.sync).dma_start(
                    out=out_ap[r0 : r0 + SUPER].rearrange(
                        "(p t) c -> p t c", p=BLOCK
                    ),
                    in_=out_t.rearrange("p (t c) -> p t c", c=3),
                )
                return

            # sigmoid(rgb) per channel -> dense per-channel tiles (ScalarE)
            if "sigmoid" in skip:
                g_views = [rgbo4[:, :, :, c] for c in range(3)]
            else:
                g_views = []
                for c in range(3):
                    g_c = p_g.tile([BLOCK, S * T], F32, tag=f"g{c}")
                    nc.scalar.activation(
                        g_c.rearrange("p (t s) -> p t s", t=T),
                        rgbo4[:, :, :, c],
                        mybir.ActivationFunctionType.Sigmoid,
                    )
                    g_views.append(g_c.rearrange("p (t s) -> p t s", t=T))

            if "dm" in skip:
                m_t = depth_t
            else:
                eng_dm = nc.gpsimd if gpsimd_delta_m else nc.vector
                delta_t = p_mid.tile([BLOCK, S * T], F32, tag="delta")
                delta3 = delta_t.rearrange("p (t s) -> p t s", t=T)
                eng_dm.tensor_sub(
                    delta3[:, :, 0 : S - 1], depth3[:, :, 1:S], depth3[:, :, 0 : S - 1]
                )
                m_t = p_mid.tile([BLOCK, S * T], F32, tag="m")
                m3 = m_t.rearrange("p (t s) -> p t s", t=T)
                eng_dm.tensor_mul(
                    m3[:, :, 0 : S - 1],
                    delta3[:, :, 0 : S - 1],
                    rgbo4[:, :, 0 : S - 1, 3],
                )
                eng_dm.tensor_scalar_mul(
                    m3[:, :, S - 1], rgbo4[:, :, S - 1, 3], 1.0e9
                )

            if "scan" in skip:
                cs_t = m_t
            else:
                cs_t = p_mid.tile([BLOCK, S * T], F32, tag="cs")
                for t in range(T):
                    nc.vector.tensor_tensor_scan(
                        cs_t[:, t * S : (t + 1) * S],
                        m_t[:, t * S : (t + 1) * S],
                        m_t[:, t * S : (t + 1) * S],
                        0.0,
                        mybir.AluOpType.add,
                        mybir.AluOpType.bypass,
                    )

            te_t = p_mid.tile([BLOCK, U * T], F32, tag="te")
            te3 = te_t.rearrange("p (t u) -> p t u", t=T)
            nc.vector.memset(te3[:, :, 0:1], 1.0)
            nc.scalar.activation(
                te3[:, :, 1 : S + 1],
                cs_t.rearrange("p (t s) -> p t s", t=T),
                mybir.ActivationFunctionType.Exp,
                scale=-1.0,
            )

            if "w" in skip:
                w_t = te_t
                w_block = lambda t: w_t[:, t * U : t * U + S]
            else:
                w_t = p_mid.tile([BLOCK, S * T], F32, tag="w")
                w3 = w_t.rearrange("p (t s) -> p t s", t=T)
                nc.vector.tensor_sub(w3, te3[:, :, 0:S], te3[:, :, 1 : S + 1])
                w_block = lambda t: w_t[:, t * S : (t + 1) * S]

            out_t = p_out.tile([BLOCK, 3 * T], F32, tag="out")
            if "stt" in skip:
                nc.vector.memset(out_t[:], 0.0)
            else:
                for t in range(T):
                    for c in range(3):
                        acc = out_t[:, t * 3 + c : t * 3 + c + 1]
                        eng = nc.vector if c < 3 - gpsimd_channels else nc.gpsimd
                        tag = "scr" if c < 3 - gpsimd_channels else "scrg"
                        scr = p_scr.tile([BLOCK, S], F32, tag=tag)
                        eng.scalar_tensor_tensor(
                            out=scr[:],
                            in0=w_block(t),
                            scalar=0.0,
                            in1=g_views[c][:, t],
                            op0=mybir.AluOpType.bypass,
                            op1=mybir.AluOpType.mult,
                            accum_out=acc,
                        )
            (nc.gpsimd if dma_split else nc.sync).dma_start(
                out=out_ap[r0 : r0 + SUPER].rearrange("(p t) c -> p t c", p=BLOCK),
                in_=out_t.rearrange("p (t c) -> p t c", c=3),
            )

        def emit_all():
            for sb in range(n_super * repeat):
                emit_superblock((sb % n_super) * SUPER)

        if loop_iters:
            with tc.For_i(0, loop_iters, 1) as _i:
                emit_all()
        else:
            emit_all()
    nc.compile()
    return nc


def build_nerf_bass_v2(
    n_rays: int = NC_RAYS,
    t_blocks: int = 8,
    bufs: int = 2,
    prod_gpsimd: bool = False,
    pad_gpsimd: bool = True,
    loop_iters: int = 0,
    dma_split: bool = False,
    delta_m_pool: bool = False,
    w_pool: bool = False,
    bf16: bool = False,
    reduce_fold: int = 0,
    skip: tuple = (),
) -> bass.Bass:
    """v2: single-activation-table (tanh+exp), masked single-instruction scan,
    broadcast product + segmented tensor_reduce.

    Math: sigmoid(x) = 0.5*tanh(x/2) + 0.5.  With te'_k = 0.5*T_k
    (te'_0 = 0.5 memset; te'_k = exp(-cs_{k-1} + ln 0.5) on ScalarE):
      out_c = sum_{i=0..S-1} (te'_i - te'_{i+1}) * tanh(rgb_ic/2) + (0.5 - te'_S)
    The additive term rides along as virtual sample S: g pad = 1.0,
    w pad = 0.5 - te'_S, so one fused product+reduce over S+1 covers it.
    The per-ray cumsum reset uses a 0/1 mask with scan op0=mult:
      state = mask*state + m  ->  segmented cumsum in ONE scan instruction.
    """
    T = t_blocks
    SUPER = BLOCK * T
    assert n_rays % SUPER == 0
    n_super = n_rays // SUPER
    S1 = S + 1           # padded sample dim for g/w (129)
    U = S + 4            # padded te stride (129 used)
    LN_HALF = -0.6931471805599453

    nc = bacc.Bacc("TRN2", target_bir_lowering=False, debug=False)
    rgbo_h = nc.declare_dram_parameter("rgbo", [n_rays, S, 4], F32, isOutput=False)
    depth_h = nc.declare_dram_parameter("depth", [n_rays, S], F32, isOutput=False)
    out_h = nc.declare_dram_parameter("out", [n_rays, 3], F32, isOutput=True)
    rgbo_ap = rgbo_h.ap()
    depth_ap = depth_h.ap()
    out_ap = out_h.ap()

    with ExitStack() as ctx:
        tc = ctx.enter_context(tile.TileContext(nc))
        p_const = ctx.enter_context(tc.tile_pool(name="const", bufs=1))
        p_rgbo = ctx.enter_context(tc.tile_pool(name="rgbo", bufs=bufs))
        p_depth = ctx.enter_context(tc.tile_pool(name="depth", bufs=bufs))
        p_g = ctx.enter_context(tc.tile_pool(name="g", bufs=bufs))
        p_mid = ctx.enter_context(tc.tile_pool(name="mid", bufs=bufs))
        p_out = ctx.enter_context(tc.tile_pool(name="outp", bufs=bufs))

        # 0/1 scan-reset mask: 0 at s=0 of each t-block, 1 elsewhere
        mask_t = p_const.tile([BLOCK, T * S], F32, tag="mask")
        mask3 = mask_t.rearrange("p (t s) -> p t s", t=T)
        nc.vector.memset(mask_t[:], 1.0)
        nc.vector.memset(mask3[:, :, 0:1], 0.0)
        # per-partition ln(0.5) bias column for the exp activation
        bias_t = p_const.tile([BLOCK, 1], F32, tag="lnhalf")
        nc.vector.memset(bias_t[:], LN_HALF)

        def emit_superblock(r0):
            rgbo_t = p_rgbo.tile([BLOCK, 4 * S * T], F32, tag="rgbo")
            rgbo_dst = rgbo_t.rearrange("p (t f) -> p t f", t=T)
            rgbo_src = rgbo_ap[r0 : r0 + SUPER].rearrange(
                "(p t) s c -> p t (s c)", p=BLOCK
            )
            depth_t = p_depth.tile([BLOCK, S * T], F32, tag="depth")
            depth_dst = depth_t.rearrange("p (t s) -> p t s", t=T)
            depth_src = depth_ap[r0 : r0 + SUPER].rearrange(
                "(p t) s -> p t s", p=BLOCK
            )
            if dma_split:
                h = T // 2
                nc.sync.dma_start(out=rgbo_dst[:, 0:h], in_=rgbo_src[:, 0:h])
                nc.scalar.dma_start(out=rgbo_dst[:, h:T], in_=rgbo_src[:, h:T])
                nc.gpsimd.dma_start(out=depth_dst, in_=depth_src)
            else:
                nc.sync.dma_start(out=rgbo_dst, in_=rgbo_src)
                nc.sync.dma_start(out=depth_dst, in_=depth_src)
            rgbo4 = rgbo_t.rearrange("p (t s c) -> p t s c", t=T, s=S, c=4)
            depth3 = depth_t.rearrange("p (t s) -> p t s", t=T)

            eng_pad = nc.gpsimd if pad_gpsimd else nc.vector
            eng_dm = nc.gpsimd if delta_m_pool else nc.vector
            eng_w = nc.gpsimd if w_pool else nc.vector
            GDT = BF16 if bf16 else F32

            # g: tanh(rgb/2) per channel, pad g[:, :, :, S] = 1.0
            g_t = p_g.tile([BLOCK, T * 3 * S1], GDT, tag="g")
            g4 = g_t.rearrange("p (t c s) -> p t c s", t=T, c=3)
            if "tanh" not in skip:
                for c in range(3):
                    nc.scalar.activation(
                        g4[:, :, c, 0:S],
                        rgbo4[:, :, :, c],
                        mybir.ActivationFunctionType.Tanh,
                        scale=0.5,
                    )
            eng_pad.memset(g4[:, :, :, S:S1], 1.0)

            # delta: diff(depth) with FAR sentinel at S-1
            if "dm" in skip:
                m_t = depth_t
            else:
                delta_t = p_mid.tile([BLOCK, S * T], F32, tag="delta")
                delta3 = delta_t.rearrange("p (t s) -> p t s", t=T)
                eng_pad.memset(delta3[:, :, S - 1 : S], FAR)
                eng_dm.tensor_sub(
                    delta3[:, :, 0 : S - 1], depth3[:, :, 1:S], depth3[:, :, 0 : S - 1]
                )
                # m = opacity * delta
                m_t = p_mid.tile([BLOCK, S * T], F32, tag="m")
                m3 = m_t.rearrange("p (t s) -> p t s", t=T)
                eng_dm.tensor_mul(m3, delta3, rgbo4[:, :, :, 3])
            # segmented inclusive cumsum in one scan: state = mask*state + m
            if "scan" in skip:
                cs_t = m_t
            else:
                cs_t = p_mid.tile([BLOCK, S * T], F32, tag="cs")
                nc.vector.tensor_tensor_scan(
                    cs_t[:],
                    mask_t[:],
                    m_t[:],
                    0.0,
                    mybir.AluOpType.mult,
                    mybir.AluOpType.add,
                )
            # te' table: te'_0 = 0.5, te'_k = exp(-cs_{k-1} + ln 0.5)
            te_t = p_mid.tile([BLOCK, U * T], F32, tag="te")
            te3 = te_t.rearrange("p (t u) -> p t u", t=T)
            eng_pad.memset(te3[:, :, 0:1], 0.5)
            if "exp" not in skip:
                nc.scalar.activation(
                    te3[:, :, 1 : S + 1],
                    cs_t.rearrange("p (t s) -> p t s", t=T),
                    mybir.ActivationFunctionType.Exp,
                    scale=-1.0,
                    bias=bias_t[:],
                )
            # w' = te'_i - te'_{i+1}; pad w'_S = 0.5 - te'_S
            w_t = p_mid.tile([BLOCK, S1 * T], GDT, tag="w")
            w3 = w_t.rearrange("p (t s) -> p t s", t=T)
            if "w" not in skip:
                eng_w.tensor_sub(w3[:, :, 0:S], te3[:, :, 0:S], te3[:, :, 1 : S + 1])
            nc.vector.tensor_scalar(
                w3[:, :, S:S1],
                te3[:, :, S : S + 1],
                -1.0,
                0.5,
                mybir.AluOpType.mult,
                mybir.AluOpType.add,
            )
            # prod = g * w (w broadcast over channel), then segmented reduce
            prod_t = p_g.tile([BLOCK, T * 3 * S1], GDT, tag="prod")
            prod4 = prod_t.rearrange("p (t c s) -> p t c s", t=T, c=3)
            w_b = w3.unsqueeze(2).broadcast_to((BLOCK, T, 3, S1))
            if "prod" not in skip:
                (nc.gpsimd if prod_gpsimd else nc.vector).tensor_mul(prod4, g4, w_b)
            # fold-add halves (bf16 TT runs at 2x) before the final reduce
            n_red = S1
            for _ in range(reduce_fold):
                h = n_red // 2  # fold [h2:n_red] onto [0:h], keep middle
                h2 = n_red - h
                nc.vector.tensor_add(
                    prod4[:, :, :, 0:h], prod4[:, :, :, 0:h], prod4[:, :, :, h2:n_red]
                )
                n_red = h2
            out_t = p_out.tile([BLOCK, 3 * T], F32, tag="out")
            if "reduce" in skip:
                nc.vector.memset(out_t[:], 0.0)
            else:
                nc.vector.tensor_reduce(
                    out_t[:],
                    prod_t.rearrange("p (tc s) -> p tc s", s=S1)[:, :, 0:n_red],
                    mybir.AxisListType.X,
                    mybir.AluOpType.add,
                )
            (nc.gpsimd if dma_split else nc.sync).dma_start(
                out=out_ap[r0 : r0 + SUPER].rearrange("(p t) c -> p t c", p=BLOCK),
                in_=out_t.rearrange("p (t c) -> p t c", c=3),
            )

        def emit_all():
            for sb in range(n_super):
                emit_superblock(sb * SUPER)

        if loop_iters:
            with tc.For_i(0, loop_iters, 1) as _i:
                emit_all()
        else:
            emit_all()
    nc.compile()
    return nc


def build_nerf_bass_v4(
    n_rays: int = NC_RAYS,
    t_blocks: int = 8,
    delta_m_pool: bool = True,
    w_pool: bool = False,
    bf16: bool = True,
    reduce_fold: int = 2,
    prod_pool: bool = False,
    loop_iters: int = 0,
    g_bufs: int = 5,
    io_bufs: int = 3,
    mid_bufs: int = 3,
) -> bass.Bass:
    """v4 = v2 math + SOFTWARE-PIPELINED emission.

    The per-superblock chain DMA->(Pool delta,m)->DVE scan->Act exp->DVE
    w,prod,reduce->DMA is too long to hide with bufs=2 program-order
    emission: engines run their instruction queues in order, so sb i+1's
    ready work sits behind sb i's not-yet-ready work. Here each tick emits
    stage k of superblock t-k in REVERSED stage order (deepest first), so
    every engine's queue interleaves superblocks and always has ready work.
    """
    T = t_blocks
    SUPER = BLOCK * T
    assert n_rays % SUPER == 0
    n_super = n_rays // SUPER
    S1 = S + 1
    U = S + 4
    LN_HALF = -0.6931471805599453
    GDT = BF16 if bf16 else F32

    nc = bacc.Bacc("TRN2", target_bir_lowering=False, debug=False)
    rgbo_h = nc.declare_dram_parameter("rgbo", [n_rays, S, 4], F32, isOutput=False)
    depth_h = nc.declare_dram_parameter("depth", [n_rays, S], F32, isOutput=False)
    out_h = nc.declare_dram_parameter("out", [n_rays, 3], F32, isOutput=True)
    rgbo_ap = rgbo_h.ap()
    depth_ap = depth_h.ap()
    out_ap = out_h.ap()

    with ExitStack() as ctx:
        tc = ctx.enter_context(tile.TileContext(nc))
        p_const = ctx.enter_context(tc.tile_pool(name="const", bufs=1))
        p_rgbo = ctx.enter_context(tc.tile_pool(name="rgbo", bufs=io_bufs))
        p_depth = ctx.enter_context(tc.tile_pool(name="depth", bufs=io_bufs))
        p_g = ctx.enter_context(tc.tile_pool(name="g", bufs=g_bufs))
        p_dm = ctx.enter_context(tc.tile_pool(name="dm", bufs=mid_bufs))
        p_cs = ctx.enter_context(tc.tile_pool(name="cs", bufs=mid_bufs))
        p_te = ctx.enter_context(tc.tile_pool(name="te", bufs=mid_bufs))
        p_w = ctx.enter_context(tc.tile_pool(name="w", bufs=2))
        p_prod = ctx.enter_context(
            tc.tile_pool(name="prod", bufs=4 if split3 else (3 if split_s4 else 2))
        )
        p_out = ctx.enter_context(tc.tile_pool(name="outp", bufs=io_bufs))

        mask_t = p_const.tile([BLOCK, T * S], F32, tag="mask")
        mask3 = mask_t.rearrange("p (t s) -> p t s", t=T)
        nc.vector.memset(mask_t[:], 1.0)
        nc.vector.memset(mask3[:, :, 0:1], 0.0)
        bias_t = p_const.tile([BLOCK, 1], F32, tag="lnhalf")
        nc.vector.memset(bias_t[:], LN_HALF)

        eng_dm = nc.gpsimd if delta_m_pool else nc.vector
        eng_w = nc.gpsimd if w_pool else nc.vector
        eng_prod = nc.gpsimd if prod_pool else nc.vector

        state: dict = {}

        def s0_dma_in(sb):
            r0 = sb * SUPER
            st = state.setdefault(sb, {})
            rgbo_t = p_rgbo.tile([BLOCK, 4 * S * T], F32, tag="rgbo", name="rgbo_t")
            nc.sync.dma_start(
                out=rgbo_t.rearrange("p (t f) -> p t f", t=T),
                in_=rgbo_ap[r0 : r0 + SUPER].rearrange("(p t) s c -> p t (s c)", p=BLOCK),
            )
            depth_t = p_depth.tile([BLOCK, S * T], F32, tag="depth", name="depth_t")
            nc.sync.dma_start(
                out=depth_t.rearrange("p (t s) -> p t s", t=T),
                in_=depth_ap[r0 : r0 + SUPER].rearrange("(p t) s -> p t s", p=BLOCK),
            )
            st["rgbo"], st["depth"] = rgbo_t, depth_t

        def s1_front(sb):
            st = state[sb]
            rgbo4 = st["rgbo"].rearrange("p (t s c) -> p t s c", t=T, s=S, c=4)
            depth3 = st["depth"].rearrange("p (t s) -> p t s", t=T)
            g_t = p_g.tile([BLOCK, T * 3 * S1], GDT, tag="g", name="g_t")
            g4 = g_t.rearrange("p (t c s) -> p t c s", t=T, c=3)
            for c in range(3):
                nc.scalar.activation(
                    g4[:, :, c, 0:S],
                    rgbo4[:, :, :, c],
                    mybir.ActivationFunctionType.Tanh,
                    scale=0.5,
                )
            nc.gpsimd.memset(g4[:, :, :, S:S1], 1.0)
            delta_t = p_dm.tile([BLOCK, S * T], F32, tag="delta", name="delta_t")
            delta3 = delta_t.rearrange("p (t s) -> p t s", t=T)
            nc.gpsimd.memset(delta3[:, :, S - 1 : S], FAR)
            eng_dm.tensor_sub(
                delta3[:, :, 0 : S - 1], depth3[:, :, 1:S], depth3[:, :, 0 : S - 1]
            )
            m_t = p_dm.tile([BLOCK, S * T], F32, tag="m", name="m_t")
            eng_dm.tensor_mul(
                m_t.rearrange("p (t s) -> p t s", t=T), delta3, rgbo4[:, :, :, 3]
            )
            st["g"], st["m"] = g_t, m_t

        def s2_scan(sb):
            st = state[sb]
            if "scan" in skip:
                st["cs"] = st["m"]
                return
            cs_t = p_cs.tile([BLOCK, S * T], F32, tag="cs", name="cs_t")
            nc.vector.tensor_tensor_scan(
                cs_t[:],
                mask_t[:],
                st["m"][:],
                0.0,
                mybir.AluOpType.mult,
                mybir.AluOpType.add,
            )
            st["cs"] = cs_t

        def s3_exp(sb):
            st = state[sb]
            te_t = p_te.tile([BLOCK, U * T], F32, tag="te", name="te_t")
            te3 = te_t.rearrange("p (t u) -> p t u", t=T)
            nc.gpsimd.memset(te3[:, :, 0:1], 0.5)
            nc.scalar.activation(
                te3[:, :, 1 : S + 1],
                st["cs"].rearrange("p (t s) -> p t s", t=T),
                mybir.ActivationFunctionType.Tanh
                if fake_exp
                else mybir.ActivationFunctionType.Exp,
                scale=-1.0,
                bias=bias_t[:],
            )
            st["te"] = te_t

        def s4_back(sb):
            st = state[sb]
            te3 = st["te"].rearrange("p (t u) -> p t u", t=T)
            g4 = st["g"].rearrange("p (t c s) -> p t c s", t=T, c=3)
            w_t = p_w.tile([BLOCK, S1 * T], GDT, tag="w", name="w_t")
            w3 = w_t.rearrange("p (t s) -> p t s", t=T)
            eng_w.tensor_sub(w3[:, :, 0:S], te3[:, :, 0:S], te3[:, :, 1 : S + 1])
            nc.vector.tensor_scalar(
                w3[:, :, S:S1],
                te3[:, :, S : S + 1],
                -1.0,
                0.5,
                mybir.AluOpType.mult,
                mybir.AluOpType.add,
            )
            prod_t = p_prod.tile([BLOCK, T * 3 * S1], GDT, tag="prod", name="prod_t")
            prod4 = prod_t.rearrange("p (t c s) -> p t c s", t=T, c=3)
            w_b = w3.unsqueeze(2).broadcast_to((BLOCK, T, 3, S1))
            eng_prod.tensor_mul(prod4, g4, w_b)
            n_red = S1
            for _ in range(reduce_fold):
                h = n_red // 2
                h2 = n_red - h
                nc.vector.tensor_add(
                    prod4[:, :, :, 0:h], prod4[:, :, :, 0:h], prod4[:, :, :, h2:n_red]
                )
                n_red = h2
            out_t = p_out.tile([BLOCK, 3 * T], F32, tag="out", name="out_t")
            if "reduce" in skip:
                nc.vector.memset(out_t[:], 0.0)
            else:
                nc.vector.tensor_reduce(
                    out_t[:],
                    prod_t.rearrange("p (tc s) -> p tc s", s=S1)[:, :, 0:n_red],
                    mybir.AxisListType.X,
                    mybir.AluOpType.add,
                )
            st["out"] = out_t

        def s5_dma_out(sb):
            st = state.pop(sb)
            r0 = sb * SUPER
            (nc.gpsimd if out_dma_pool else nc.sync).dma_start(
                out=out_ap[r0 : r0 + SUPER].rearrange("(p t) c -> p t c", p=BLOCK),
                in_=st["out"].rearrange("p (t c) -> p t c", c=3),
            )

        if split3:
            stages = [
                s0_dma_in, s1_front, s2_scan, s3_exp, s4_back, s4b1_fold,
                s4b_reduce, s5_dma_out,
            ]
        elif split_s4:
            stages = [
                s0_dma_in, s1_front, s2_scan, s3_exp, s4_back, s4b_reduce, s5_dma_out,
            ]
        else:
            stages = [s0_dma_in, s1_front, s2_scan, s3_exp, s4_back, s5_dma_out]
        NS = len(stages)

        def emit_all():
            for tick in range(n_super + NS - 1):
                for si in reversed(range(NS)):
                    sb = tick - si
                    if 0 <= sb < n_super:
                        stages[si](sb)

        if loop_iters:
            with tc.For_i(0, loop_iters, 1) as _i:
                emit_all()
        else:
            emit_all()
    nc.compile()
    return nc


def build_nerf_bass_v5(
    n_rays: int = NC_RAYS,
    t_blocks: int = 8,
    delta_m_pool: bool = True,
    loop_iters: int = 0,
    g_bufs: int = 5,
    io_bufs: int = 3,
    mid_bufs: int = 3,
    reduce_fold: int = 2,
    te_bf16: bool = True,
    fake_exp: bool = False,
    skip: tuple = (),
    split_s4: bool = False,
    split3: bool = False,
    out_dma_pool: bool = False,
) -> bass.Bass:
    """v5 = v4 software pipeline + persistent pad tiles + bf16 te.

    g/delta/te pad columns are initialized ONCE (they are never overwritten
    by the per-superblock compute), so no per-sb pad memsets sit in any
    engine queue. te in bf16 makes the w subtraction all-bf16 (4x DVE mode).
    Pool (if enabled) runs ONLY delta+m so its in-order queue never blocks
    the te/exp path.
    """
    T = t_blocks
    SUPER = BLOCK * T
    assert n_rays % SUPER == 0
    n_super = n_rays // SUPER
    S1 = S + 1
    U = S + 4
    LN_HALF = -0.6931471805599453
    GDT = BF16
    TDT = BF16 if te_bf16 else F32

    nc = bacc.Bacc("TRN2", target_bir_lowering=False, debug=False)
    rgbo_h = nc.declare_dram_parameter("rgbo", [n_rays, S, 4], F32, isOutput=False)
    depth_h = nc.declare_dram_parameter("depth", [n_rays, S], F32, isOutput=False)
    out_h = nc.declare_dram_parameter("out", [n_rays, 3], F32, isOutput=True)
    rgbo_ap = rgbo_h.ap()
    depth_ap = depth_h.ap()
    out_ap = out_h.ap()

    with ExitStack() as ctx:
        tc = ctx.enter_context(tile.TileContext(nc))
        p_const = ctx.enter_context(tc.tile_pool(name="const", bufs=1))
        p_rgbo = ctx.enter_context(tc.tile_pool(name="rgbo", bufs=io_bufs))
        p_depth = ctx.enter_context(tc.tile_pool(name="depth", bufs=io_bufs))
        p_m = ctx.enter_context(tc.tile_pool(name="m", bufs=mid_bufs))
        p_cs = ctx.enter_context(tc.tile_pool(name="cs", bufs=mid_bufs))
        p_w = ctx.enter_context(tc.tile_pool(name="w", bufs=2))
        p_prod = ctx.enter_context(
            tc.tile_pool(name="prod", bufs=4 if split3 else (3 if split_s4 else 2))
        )
        p_out = ctx.enter_context(tc.tile_pool(name="outp", bufs=io_bufs))

        mask_t = p_const.tile([BLOCK, T * S], F32, tag="mask")
        mask3 = mask_t.rearrange("p (t s) -> p t s", t=T)
        nc.vector.memset(mask_t[:], 1.0)
        nc.vector.memset(mask3[:, :, 0:1], 0.0)
        bias_t = p_const.tile([BLOCK, 1], F32, tag="lnhalf")
        nc.vector.memset(bias_t[:], LN_HALF)

        # persistent rotated tiles with one-time pad init
        g_tiles = []
        for i in range(g_bufs):
            gt = p_const.tile([BLOCK, T * 3 * S1], GDT, tag=f"g{i}", name=f"g{i}")
            nc.vector.memset(
                gt.rearrange("p (t c s) -> p t c s", t=T, c=3)[:, :, :, S:S1], 1.0
            )
            g_tiles.append(gt)
        delta_tiles = []
        for i in range(mid_bufs):
            dt = p_const.tile([BLOCK, S * T], F32, tag=f"delta{i}", name=f"delta{i}")
            nc.vector.memset(
                dt.rearrange("p (t s) -> p t s", t=T)[:, :, S - 1 : S], FAR
            )
            delta_tiles.append(dt)
        te_tiles = []
        for i in range(mid_bufs):
            tt = p_const.tile([BLOCK, U * T], TDT, tag=f"te{i}", name=f"te{i}")
            nc.vector.memset(
                tt.rearrange("p (t u) -> p t u", t=T)[:, :, 0:1], 0.5
            )
            te_tiles.append(tt)

        eng_dm = nc.gpsimd if delta_m_pool else nc.vector
        state: dict = {}

        def s0_dma_in(sb):
            r0 = sb * SUPER
            st = state.setdefault(sb, {})
            depth_t = p_depth.tile([BLOCK, S * T], F32, tag="depth", name="depth_t")
            nc.sync.dma_start(
                out=depth_t.rearrange("p (t s) -> p t s", t=T),
                in_=depth_ap[r0 : r0 + SUPER].rearrange("(p t) s -> p t s", p=BLOCK),
            )
            rgbo_t = p_rgbo.tile([BLOCK, 4 * S * T], F32, tag="rgbo", name="rgbo_t")
            nc.sync.dma_start(
                out=rgbo_t.rearrange("p (t f) -> p t f", t=T),
                in_=rgbo_ap[r0 : r0 + SUPER].rearrange(
                    "(p t) s c -> p t (s c)", p=BLOCK
                ),
            )
            st["rgbo"], st["depth"] = rgbo_t, depth_t

        def s1_front(sb):
            st = state[sb]
            rgbo4 = st["rgbo"].rearrange("p (t s c) -> p t s c", t=T, s=S, c=4)
            depth3 = st["depth"].rearrange("p (t s) -> p t s", t=T)
            g_t = g_tiles[sb % g_bufs]
            g4 = g_t.rearrange("p (t c s) -> p t c s", t=T, c=3)
            if "tanh" not in skip:
                for c in range(3):
                    nc.scalar.activation(
                        g4[:, :, c, 0:S],
                        rgbo4[:, :, :, c],
                        mybir.ActivationFunctionType.Tanh,
                        scale=0.5,
                    )
            if "dm" in skip:
                m_t = st["depth"]
            else:
                delta_t = delta_tiles[sb % mid_bufs]
                delta3 = delta_t.rearrange("p (t s) -> p t s", t=T)
                eng_dm.tensor_sub(
                    delta3[:, :, 0 : S - 1], depth3[:, :, 1:S], depth3[:, :, 0 : S - 1]
                )
                m_t = p_m.tile([BLOCK, S * T], F32, tag="m", name="m_t")
                eng_dm.tensor_mul(
                    m_t.rearrange("p (t s) -> p t s", t=T), delta3, rgbo4[:, :, :, 3]
                )
            st["g"], st["m"] = g_t, m_t

        def s2_scan(sb):
            st = state[sb]
            if "scan" in skip:
                st["cs"] = st["m"]
                return
            cs_t = p_cs.tile([BLOCK, S * T], F32, tag="cs", name="cs_t")
            nc.vector.tensor_tensor_scan(
                cs_t[:],
                mask_t[:],
                st["m"][:],
                0.0,
                mybir.AluOpType.mult,
                mybir.AluOpType.add,
            )
            st["cs"] = cs_t

        def s3_exp(sb):
            st = state[sb]
            te_t = te_tiles[sb % mid_bufs]
            te3 = te_t.rearrange("p (t u) -> p t u", t=T)
            if "exp" in skip:
                st["te"] = te_t
                return
            nc.scalar.activation(
                te3[:, :, 1 : S + 1],
                st["cs"].rearrange("p (t s) -> p t s", t=T),
                mybir.ActivationFunctionType.Tanh
                if fake_exp
                else mybir.ActivationFunctionType.Exp,
                scale=-1.0,
                bias=bias_t[:],
            )
            st["te"] = te_t

        def s4_back(sb):
            st = state[sb]
            te3 = st["te"].rearrange("p (t u) -> p t u", t=T)
            g4 = st["g"].rearrange("p (t c s) -> p t c s", t=T, c=3)
            w_t = p_w.tile([BLOCK, S1 * T], GDT, tag="w", name="w_t")
            w3 = w_t.rearrange("p (t s) -> p t s", t=T)
            if "w" not in skip:
                nc.vector.tensor_sub(
                    w3[:, :, 0:S], te3[:, :, 0:S], te3[:, :, 1 : S + 1]
                )
            nc.vector.tensor_scalar(
                w3[:, :, S:S1],
                te3[:, :, S : S + 1],
                -1.0,
                0.5,
                mybir.AluOpType.mult,
                mybir.AluOpType.add,
            )
            prod_t = p_prod.tile([BLOCK, T * 3 * S1], GDT, tag="prod", name="prod_t")
            prod4 = prod_t.rearrange("p (t c s) -> p t c s", t=T, c=3)
            w_b = w3.unsqueeze(2).broadcast_to((BLOCK, T, 3, S1))
            if "prod" not in skip:
                nc.vector.tensor_mul(prod4, g4, w_b)
            st["prod"] = prod_t
            if not split_s4:
                s4b_reduce(sb)

        def _fold(sb, n_folds):
            st = state[sb]
            prod4 = st["prod"].rearrange("p (t c s) -> p t c s", t=T, c=3)
            n_red = st.get("n_red", S1)
            for _ in range(n_folds):
                h = n_red // 2
                h2 = n_red - h
                nc.vector.tensor_add(
                    prod4[:, :, :, 0:h], prod4[:, :, :, 0:h], prod4[:, :, :, h2:n_red]
                )
                n_red = h2
            st["n_red"] = n_red

        def s4b1_fold(sb):
            if "reduce" not in skip:
                _fold(sb, 2)

        def s4b_reduce(sb):
            st = state[sb]
            prod_t = st["prod"]
            if "reduce" not in skip:
                if split3:
                    _fold(sb, reduce_fold - 2)
                else:
                    _fold(sb, reduce_fold)
            n_red = st.get("n_red", S1)
            out_t = p_out.tile([BLOCK, 3 * T], F32, tag="out", name="out_t")
            if "reduce" in skip:
                nc.vector.memset(out_t[:], 0.0)
            else:
                nc.vector.tensor_reduce(
                    out_t[:],
                    prod_t.rearrange("p (tc s) -> p tc s", s=S1)[:, :, 0:n_red],
                    mybir.AxisListType.X,
                    mybir.AluOpType.add,
                )
            st["out"] = out_t

        def s5_dma_out(sb):
            st = state.pop(sb)
            r0 = sb * SUPER
            (nc.gpsimd if out_dma_pool else nc.sync).dma_start(
                out=out_ap[r0 : r0 + SUPER].rearrange("(p t) c -> p t c", p=BLOCK),
                in_=st["out"].rearrange("p (t c) -> p t c", c=3),
            )

        if split3:
            stages = [
                s0_dma_in, s1_front, s2_scan, s3_exp, s4_back, s4b1_fold,
                s4b_reduce, s5_dma_out,
            ]
        elif split_s4:
            stages = [
                s0_dma_in, s1_front, s2_scan, s3_exp, s4_back, s4b_reduce, s5_dma_out,
            ]
        else:
            stages = [s0_dma_in, s1_front, s2_scan, s3_exp, s4_back, s5_dma_out]
        NS = len(stages)

        def emit_all():
            for tick in range(n_super + NS - 1):
                for si in reversed(range(NS)):
                    sb = tick - si
                    if 0 <= sb < n_super:
                        stages[si](sb)

        if loop_iters:
            with tc.For_i(0, loop_iters, 1) as _i:
                emit_all()
        else:
            emit_all()
    nc.compile()
    return nc


FAR = 1.0e9

_NC_CACHE: dict = {}


BEST_KWARGS = {"delta_m_pool": False, "split_s4": True, "reduce_fold": 3}


def _get_nc():
    if "nc" not in _NC_CACHE:
        _NC_CACHE["nc"] = build_nerf_bass_v5(**BEST_KWARGS)
    return _NC_CACHE["nc"]


def kernel(rgbo: np.ndarray, depth: np.ndarray, **run_kwargs) -> np.ndarray:
    rgbo = np.ascontiguousarray(rgbo, dtype=np.float32)
    depth = np.ascontiguousarray(depth, dtype=np.float32)
    assert rgbo.shape == (N_RAYS, S, 4) and depth.shape == (N_RAYS, S)

    nc = _get_nc()
    in_maps = []
    for i in range(N_CORES):
        sl = slice(i * NC_RAYS, (i + 1) * NC_RAYS)
        in_maps.append({"rgbo": rgbo[sl], "depth": depth[sl]})
    res = run_bass_kernel_spmd(nc, in_maps, core_ids=list(range(N_CORES)), **run_kwargs)
    out = np.concatenate([r["out"] for r in res.results], axis=0)
    if run_kwargs:
        kernel.last_results = res  # stash for profiling harnesses
    return out

